# revision 12
# baseline (speedup 1.0000x reference)
"""Causal self-attention on 8 Trainium2 NeuronCores (Bass/Tile), bf16.

Sharding: core c -> (batch b = c//2, head-group g = c%2).  Each core runs
attention for 8 heads of one batch element: qkv projection (columns of
w_qkv for its heads), causal softmax attention, and its half of the
output projection (rows of w_proj).  Host sums the two partial
projections per batch and transposes back.

Dataflow is fully transposed on-device (the contraction dim always sits
on SBUF partitions, so no on-device transposes are needed anywhere):
  xT [D, T] -> qT/kT [64, T] per head -> S^T [kidx, q] blocks -> exp ->
  PV gives out^T [d, q]; an appended ones-column in the stationary
  operand makes the PE produce the softmax denominators for free ->
  w_proj consumed natively as lhsT -> oT [D, T] partial output.

All SBUF data and DMA traffic is bf16 (PSUM accumulation stays fp32);
the host casts inputs and pre-tiles them to [128, ...] so each weight
loads in a single large DMA (HWDGE issue cost is ~0.6us per dma_start).

Per head pair the two heads are processed kt-interleaved: their QK^T
matmuls land in different PE row groups (head even uses partitions
0-63, head odd 64-127) so the array runs them concurrently, and their
PV accumulations share one PSUM bank per q-chunk -- the [V_h|J] /
[J|V_h] stationary layouts write disjoint partition rows (out rows
0-63 + sums row 96 for the even head, out rows 64-127 + sums row 32
for the odd head), so accumulating both into the same tile is exact.

Softmax skips max-subtraction (scores ~ N(0,1)); the denominators are
reciprocal'd in a [128, HW/128] reshaped tile mid-DRAM-bounce so the
DVE's 8-cycle/element iterative divide runs 128-lane-parallel, then
broadcast to the out rows via a second DRAM bounce (SBUF DMA cannot
replicate partitions).
"""

import sys

sys.path.insert(0, "/opt/trn_rl_repo")

import numpy as np

N_CORES = 8
B, T, D = 4, 2048, 1024
H, HD = 16, 64
HG = 2  # head groups (tensor parallel)
HC = H // HG  # heads per core
CD = HC * HD  # per-core qkv width (512)

_CACHE = {}


def build_attention_kernel(T, D, HC, HD, n_cores=N_CORES):
    """Build + compile the per-core Bass module (same module on all cores)."""
    from contextlib import ExitStack

    import concourse.bass as bass
    import concourse.mybir as mybir
    import concourse.tile as tile
    from concourse import bacc

    f32 = mybir.dt.float32
    b16 = mybir.dt.bfloat16
    EXP = mybir.ActivationFunctionType.Exp

    CD = HC * HD
    DT = D // 128  # contraction chunks over D
    NT = T // 128  # T tiles / kidx tiles
    CT = CD // 128  # col tiles of q/k block (= head pairs)
    HW = T // 2  # q-half width
    HWP = HW // 128  # reshaped denominator cols per partition
    NQC = HW // 512  # 512-wide q chunks per half
    PB = 3 * HD  # V_sb block width per head pair: [V_even | J | V_odd]
    VW = CT * PB  # V_sb row width
    NQ5 = T // 512
    scale = 1.0 / float(np.sqrt(HD))

    nc = bacc.Bacc("TRN2", target_bir_lowering=False, debug=False, num_devices=n_cores)

    xT_d = nc.dram_tensor("xT", [128, DT * T], b16, kind="ExternalInput").ap()
    wq_d = nc.dram_tensor("wq", [128, DT * CD], b16, kind="ExternalInput").ap()
    wk_d = nc.dram_tensor("wk", [128, DT * CD], b16, kind="ExternalInput").ap()
    wv_d = nc.dram_tensor("wv", [128, DT * CD], b16, kind="ExternalInput").ap()
    wp_d = nc.dram_tensor("wp", [128, CT * D], b16, kind="ExternalInput").ap()
    jc_d = nc.dram_tensor("jc", [128, HD], b16, kind="ExternalInput").ap()
    ut_d = nc.dram_tensor("ut", [128, 128], b16, kind="ExternalInput").ap()
    oT_d = nc.dram_tensor("oT", [D, T], b16, kind="ExternalOutput").ap()

    with tile.TileContext(nc) as tc, ExitStack() as ctx:
        # ---- persistent SBUF ----
        pers = ctx.enter_context(tc.tile_pool(name="pers", bufs=1))
        V_sb = pers.tile([128, NT, VW], b16)
        QT_sb = pers.tile([128, CT, T], b16)
        KT_sb = pers.tile([128, CT, T], b16)
        utri = pers.tile([128, 128], b16)
        nc.sync.dma_start(out=utri[:], in_=ut_d[:])

        J_sb = pers.tile([128, HD], b16)
        nc.sync.dma_start(out=J_sb[:], in_=jc_d[:])
        for kt in range(NT):
            for p_i in range(CT):
                nc.vector.tensor_copy(
                    V_sb[:, kt, p_i * PB + HD : p_i * PB + 2 * HD], J_sb[:]
                )

        # ---- phase K/Q/V: projections of x (xT resident only here) ----
        with (
            tc.tile_pool(name="pwk", bufs=1) as pwk,
            tc.tile_pool(name="px", bufs=1) as px,
        ):
            wk_sb = pwk.tile([128, DT, CD], b16)
            nc.sync.dma_start(out=wk_sb.rearrange("p a b -> p (a b)"), in_=wk_d[:])
            # xT in dt-pair tiles so the first matmuls only wait on the
            # first 1 MB, not the full 4 MB
            xts = []
            for dp in range(DT // 2):
                xt = px.tile([128, 2, T], b16, name=f"xt{dp}")
                nc.sync.dma_start(
                    out=xt.rearrange("p a b -> p (a b)"),
                    in_=xT_d[:, dp * 2 * T : (dp + 1) * 2 * T],
                )
                xts.append(xt)

            def xchunk(dt, c0, c1):
                return xts[dt // 2][:, dt % 2, c0:c1]

            # KT/QT[col, t] = w.T @ x (transposed orientation).  Loop qc inner
            # so each stationary w chunk serves T//512 matmuls.
            for w_d, T_sb, w_sb0, pname in (
                (None, KT_sb, wk_sb, "pwk"),
                (wq_d, QT_sb, None, "pwq"),
            ):
                with (
                    tc.tile_pool(name=pname + "w", bufs=1) as pwx,
                    tc.tile_pool(name=pname + "j", bufs=2 * NQ5, space="PSUM") as cprj,
                ):
                    if w_sb0 is None:
                        w_sb = pwx.tile([128, DT, CD], b16, name=pname + "t")
                        nc.sync.dma_start(out=w_sb.rearrange("p a b -> p (a b)"), in_=w_d[:])
                    else:
                        w_sb = w_sb0
                    for ct in range(CT):
                        c_ps = [
                            cprj.tile([128, 512], f32, tag="cps", name=f"{pname}ps{ct}{qc}")
                            for qc in range(NQ5)
                        ]
                        for dt in range(DT):
                            for qc in range(NQ5):
                                nc.tensor.matmul(
                                    c_ps[qc][:],
                                    w_sb[:, dt, ct * 128 : (ct + 1) * 128],
                                    xchunk(dt, qc * 512, (qc + 1) * 512),
                                    start=(dt == 0),
                                    stop=(dt == DT - 1),
                                )
                        for qc in range(NQ5):
                            nc.scalar.copy(T_sb[:, ct, qc * 512 : (qc + 1) * 512], c_ps[qc][:])

            # V[t, vcol] = x @ wv  (normal orientation: kidx on partitions)
            with (
                tc.tile_pool(name="pwv", bufs=1) as pwv,
                tc.tile_pool(name="vprj", bufs=2, space="PSUM") as vprj,
            ):
                wv_sb = pwv.tile([128, DT, CD], b16)
                nc.sync.dma_start(out=wv_sb.rearrange("p a b -> p (a b)"), in_=wv_d[:])
                for t in range(NT):
                    v_ps = vprj.tile([128, CD], f32, tag="vps")
                    for dt in range(DT):
                        nc.tensor.matmul(
                            v_ps[:],
                            xchunk(dt, t * 128, (t + 1) * 128),
                            wv_sb[:, dt, :],
                            start=(dt == 0),
                            stop=(dt == DT - 1),
                        )
                    Vv = V_sb.rearrange("p t (c w) -> p t c w", w=PB)
                    pv = v_ps.rearrange("p (c two h) -> p c two h", two=2, h=HD)
                    nc.vector.tensor_copy(Vv[:, t, :, 0:HD], pv[:, :, 0, :])
                    nc.vector.tensor_copy(Vv[:, t, :, 2 * HD : PB], pv[:, :, 1, :])

        # ---- attention (xT / w pools released) ----
        poT = ctx.enter_context(tc.tile_pool(name="poT", bufs=1))
        oT_sb = poT.tile([128, CT, T], b16)

        with (
            tc.tile_pool(name="pexp", bufs=4) as pexp,
            tc.tile_pool(name="pnrm", bufs=2) as pnrm,
            tc.tile_pool(name="pdr", bufs=4, space="DRAM") as pdr,
            tc.tile_pool(name="sps", bufs=2, space="PSUM") as sps,
            tc.tile_pool(name="aps", bufs=2 * NQC, space="PSUM") as aps,
        ):
            for p_i in range(CT):
                for hh in range(2):
                    QTh = QT_sb[hh * 64 : (hh + 1) * 64, p_i, :]
                    KTh = KT_sb[hh * 64 : (hh + 1) * 64, p_i, :]
                    # ones sits at J col HD//2: sums row = HD + HD//2 (even
                    # head, lhsT=[V_h|J]) or HD//2 (odd head, lhsT=[J|V_h])
                    sum_row = HD // 2 if hh else HD + HD // 2
                    out_lo = HD if hh else 0
                    vbase = HD if hh else 0
                    for half in range(2):
                        q0 = half * HW
                        kt_hi = (q0 + HW) // 128
                        accs = [
                            aps.tile([128, 512], f32, name=f"acc{p_i}{hh}{half}{i}", tag="acc")
                            for i in range(NQC)
                        ]

                        def emit_pv(kt, ex, lo):
                            lhsT = V_sb[:, kt, p_i * PB + vbase : p_i * PB + vbase + 2 * HD]
                            for qcl in range(NQC):
                                rlo = max(lo - qcl * 512, 0)
                                if rlo >= 512:
                                    continue  # block fully above this q chunk
                                last_kt = (q0 + (qcl + 1) * 512) // 128 - 1
                                nc.tensor.matmul(
                                    accs[qcl][:, rlo:],
                                    lhsT,
                                    ex[:, qcl * 512 + rlo : (qcl + 1) * 512],
                                    start=(kt == 0),
                                    stop=(kt == last_kt),
                                )

                        # PV trails S/exp by 2 kt steps so the PE never waits
                        # on the scalar engine's exp
                        pend = []
                        for kt in range(kt_hi):
                            lo = max(kt * 128 - q0, 0)
                            s_ps = sps.tile([128, HW], f32, tag="sps")
                            c = lo
                            while c < HW:
                                c1 = min((c // 512 + 1) * 512, HW)
                                nc.tensor.matmul(
                                    s_ps[:, c:c1],
                                    KTh[:, kt * 128 : (kt + 1) * 128],
                                    QTh[:, q0 + c : q0 + c1],
                                    start=True,
                                    stop=True,
                                )
                                c = c1
                            ex = pexp.tile([128, HW], b16, tag="ex")
                            nc.scalar.activation(ex[:, lo:], s_ps[:, lo:], EXP, scale=scale)
                            if kt * 128 >= q0:  # diagonal block lives in this half
                                dl = kt * 128 - q0
                                nc.vector.tensor_mul(ex[:, dl : dl + 128], ex[:, dl : dl + 128], utri[:])
                            pend.append((kt, ex, lo))
                            if len(pend) > 2:
                                emit_pv(*pend.pop(0))
                        for e in pend:
                            emit_pv(*e)

                        # normalize: out^T[d, q] * (1 / sums[q]).  Reciprocal
                        # is done in a [128, HW/128] reshaped tile mid-bounce
                        # (the DVE divide is 8 cyc/elem; spread it across all
                        # lanes), then broadcast to the out rows via a second
                        # DRAM bounce.
                        rb_row = pnrm.tile([1, HW], f32, name=f"rr{p_i}{hh}{half}", tag="rbrow")
                        for qcl in range(NQC):
                            nc.vector.tensor_copy(
                                rb_row[0:1, qcl * 512 : (qcl + 1) * 512],
                                accs[qcl][sum_row : sum_row + 1, :],
                            )
                        scr = pdr.tile([1, HW], f32, name=f"scr{p_i}{hh}{half}", tag="scr")
                        nc.sync.dma_start(out=scr[:], in_=rb_row[0:1, :])
                        rsh = pnrm.tile([128, HWP], f32, name=f"rs{p_i}{hh}{half}", tag="rsh")
                        scr_sq = bass.AP(tensor=scr.tensor, offset=scr.offset, ap=[[HWP, 128], [1, HWP]])
                        nc.sync.dma_start(out=rsh[:], in_=scr_sq)
                        nc.vector.reciprocal(rsh[:], rsh[:])
                        scr2 = pdr.tile([1, HW], f32, name=f"sc2{p_i}{hh}{half}", tag="scr2")
                        scr2_sq = bass.AP(tensor=scr2.tensor, offset=scr2.offset, ap=[[HWP, 128], [1, HWP]])
                        nc.sync.dma_start(out=scr2_sq, in_=rsh[:])
                        rb = pnrm.tile([128, HW], f32, name=f"rb{p_i}{hh}{half}", tag="rb")
                        sbc = bass.AP(tensor=scr2.tensor, offset=scr2.offset, ap=[[0, HD], [1, HW]])
                        nc.sync.dma_start(out=rb[out_lo : out_lo + HD, :], in_=sbc)
                        for qcl in range(NQC):
                            nc.vector.tensor_mul(
                                oT_sb[out_lo : out_lo + HD, p_i, q0 + qcl * 512 : q0 + (qcl + 1) * 512],
                                accs[qcl][out_lo : out_lo + HD, :],
                                rb[out_lo : out_lo + HD, qcl * 512 : (qcl + 1) * 512],
                            )

        # ---- output projection: oT = wp.T @ out^T ----
        with (
            tc.tile_pool(name="pwp", bufs=1) as pwp,
            tc.tile_pool(name="ppo", bufs=3) as ppo,
            tc.tile_pool(name="pps", bufs=2 * NQ5, space="PSUM") as pps,
        ):
            wp_sb = pwp.tile([128, CT, D], b16)
            nc.sync.dma_start(out=wp_sb.rearrange("p a b -> p (a b)"), in_=wp_d[:])
            for nt in range(D // 128):
                p_ps = [
                    pps.tile([128, 512], f32, tag="pps", name=f"pps{nt}{qc}")
                    for qc in range(NQ5)
                ]
                for ct in range(CT):
                    for qc in range(NQ5):
                        nc.tensor.matmul(
                            p_ps[qc][:],
                            wp_sb[:, ct, nt * 128 : (nt + 1) * 128],
                            oT_sb[:, ct, qc * 512 : (qc + 1) * 512],
                            start=(ct == 0),
                            stop=(ct == CT - 1),
                        )
                po = ppo.tile([128, T], b16, tag="po", name=f"po{nt}")
                for qc in range(NQ5):
                    nc.vector.tensor_copy(po[:, qc * 512 : (qc + 1) * 512], p_ps[qc][:])
                nc.sync.dma_start(out=oT_d[nt * 128 : (nt + 1) * 128, :], in_=po[:])

    nc.compile()
    return nc


def _get_compiled():
    key = (T, D, HC, HD)
    if key not in _CACHE:
        _CACHE[key] = build_attention_kernel(*key)
    return _CACHE[key]


def _tile128(a, chunk):
    """[R, C] -> [128, (R//128)*C] with row-block-major layout."""
    R, C = a.shape
    return np.ascontiguousarray(
        a.reshape(R // 128, 128, C).transpose(1, 0, 2).reshape(128, (R // 128) * C)
    )


def shard_inputs(x, w_qkv, w_proj):
    import ml_dtypes

    bf = ml_dtypes.bfloat16
    jc = np.zeros((128, HD), np.float32)
    jc[:, HD // 2] = 1.0
    jc = jc.astype(bf)
    ut = np.triu(np.ones((128, 128), np.float32)).astype(bf)
    in_maps = []
    for c in range(N_CORES):
        b, g = c // HG, c % HG
        in_maps.append(
            dict(
                jc=jc,
                ut=ut,
                xT=_tile128(x[b].T.astype(bf), T),
                wq=_tile128(w_qkv[:, g * CD : (g + 1) * CD].astype(bf), CD),
                wk=_tile128(w_qkv[:, D + g * CD : D + (g + 1) * CD].astype(bf), CD),
                wv=_tile128(w_qkv[:, 2 * D + g * CD : 2 * D + (g + 1) * CD].astype(bf), CD),
                wp=_tile128(w_proj[g * CD : (g + 1) * CD, :].astype(bf), D),
            )
        )
    return in_maps


def gather_outputs(results):
    out = np.empty((B, T, D), np.float32)
    for b in range(B):
        acc = results[HG * b]["oT"].astype(np.float32) + results[HG * b + 1]["oT"].astype(np.float32)
        out[b] = acc.T
    return out


def kernel(x, w_qkv, w_proj):
    from concourse.bass_utils import run_bass_kernel_spmd

    x = np.asarray(x, dtype=np.float32)
    w_qkv = np.asarray(w_qkv, dtype=np.float32)
    w_proj = np.asarray(w_proj, dtype=np.float32)
    nc = _get_compiled()
    res = run_bass_kernel_spmd(nc, shard_inputs(x, w_qkv, w_proj), list(range(N_CORES)))
    return gather_outputs(res.results)


# revision 17
# speedup vs baseline: 1.2013x; 1.2013x over previous
"""Causal self-attention on 8 Trainium2 NeuronCores (Bass/Tile), bf16.

Sharding: core c -> (batch b = c//2, head-group g = c%2).  Each core runs
attention for 8 heads of one batch element: qkv projection (columns of
w_qkv for its heads), causal softmax attention, and its half of the
output projection (rows of w_proj).  Host sums the two partial
projections per batch and transposes back.

Dataflow is fully transposed on-device (the contraction dim always sits
on SBUF partitions, so no on-device transposes are needed anywhere):
  xT [D, T] -> qT/kT [64, T] per head -> S^T [kidx, q] blocks -> exp ->
  PV gives out^T [d, q]; an appended ones-column in the stationary
  operand makes the PE produce the softmax denominators for free ->
  w_proj consumed natively as lhsT -> oT [D, T] partial output.

All SBUF data and DMA traffic is bf16 (PSUM accumulation stays fp32);
the host casts inputs and pre-tiles them to [128, ...] so each weight
loads in a single large DMA (HWDGE issue cost is ~0.6us per dma_start).

V layout per kidx-tile and head pair: [V_even | J | V_odd] where J is
64 cols of zeros with a 1.0 in col 32.  PV stationary for an even head
is [V_h | J] (out rows 0-63, sums row 96), for an odd head [J | V_h]
(sums row 32, out rows 64-127) -- the appended ones-column makes the PE
produce the softmax denominators in the same pass as PV, and keeps all
later elementwise ops partition- and 32-aligned.

Softmax skips max-subtraction (scores ~ N(0,1); exp is safe in fp32)
and causal-skips fully-masked blocks in scores, exp, and PV.  The
denominators are reciprocal'd in a [128, HW/128] reshaped tile
mid-DRAM-bounce so the DVE's 8-cycle/element iterative divide runs
128-lane-parallel, then broadcast to the out rows via a second DRAM
bounce (SBUF DMA cannot replicate partitions).
"""

import sys

sys.path.insert(0, "/opt/trn_rl_repo")

import numpy as np

N_CORES = 8
B, T, D = 4, 2048, 1024
H, HD = 16, 64
HG = 2  # head groups (tensor parallel)
HC = H // HG  # heads per core
CD = HC * HD  # per-core qkv width (512)

_CACHE = {}


def build_attention_kernel(T, D, HC, HD, n_cores=N_CORES):
    """Build + compile the per-core Bass module (same module on all cores)."""
    from contextlib import ExitStack

    import concourse.bass as bass
    import concourse.mybir as mybir
    import concourse.tile as tile
    from concourse import bacc

    f32 = mybir.dt.float32
    b16 = mybir.dt.bfloat16
    EXP = mybir.ActivationFunctionType.Exp

    CD = HC * HD
    DT = D // 128  # contraction chunks over D
    NT = T // 128  # T tiles / kidx tiles
    CT = CD // 128  # col tiles of q/k block (= head pairs)
    HW = T // 2  # q-half width
    HWP = HW // 128  # reshaped denominator cols per partition
    NQC = HW // 512  # 512-wide q chunks per half
    PB = 3 * HD  # V_sb block width per head pair: [V_even | J | V_odd]
    VW = CT * PB  # V_sb row width
    NQ5 = T // 512
    scale = 1.0 / float(np.sqrt(HD))

    nc = bacc.Bacc("TRN2", target_bir_lowering=False, debug=False, num_devices=n_cores)

    xT_d = nc.dram_tensor("xT", [128, DT * T], b16, kind="ExternalInput").ap()
    wq_d = nc.dram_tensor("wq", [128, DT * CD], b16, kind="ExternalInput").ap()
    wk_d = nc.dram_tensor("wk", [128, DT * CD], b16, kind="ExternalInput").ap()
    wv_d = nc.dram_tensor("wv", [128, DT * CD], b16, kind="ExternalInput").ap()
    wp_d = nc.dram_tensor("wp", [128, CT * D], b16, kind="ExternalInput").ap()
    jc_d = nc.dram_tensor("jc", [128, HD], b16, kind="ExternalInput").ap()
    ut_d = nc.dram_tensor("ut", [128, 128], b16, kind="ExternalInput").ap()
    oT_d = nc.dram_tensor("oT", [D, T], b16, kind="ExternalOutput").ap()

    with tile.TileContext(nc) as tc, ExitStack() as ctx:
        # ---- persistent SBUF ----
        pers = ctx.enter_context(tc.tile_pool(name="pers", bufs=1))
        V_sb = pers.tile([128, NT, VW], b16)
        QT_sb = pers.tile([128, CT, T], b16)
        KT_sb = pers.tile([128, CT, T], b16)
        utri = pers.tile([128, 128], b16)
        nc.sync.dma_start(out=utri[:], in_=ut_d[:])

        J_sb = pers.tile([128, HD], b16)
        nc.sync.dma_start(out=J_sb[:], in_=jc_d[:])
        for kt in range(NT):
            for p_i in range(CT):
                nc.vector.tensor_copy(
                    V_sb[:, kt, p_i * PB + HD : p_i * PB + 2 * HD], J_sb[:]
                )

        # ---- phase K/Q/V: projections of x (xT resident only here) ----
        with (
            tc.tile_pool(name="pwk", bufs=1) as pwk,
            tc.tile_pool(name="px", bufs=1) as px,
        ):
            wk_sb = pwk.tile([128, DT, CD], b16)
            nc.sync.dma_start(out=wk_sb.rearrange("p a b -> p (a b)"), in_=wk_d[:])
            # xT in dt-pair tiles so the first matmuls only wait on the
            # first 1 MB, not the full 4 MB
            xts = []
            for dp in range(DT // 2):
                xt = px.tile([128, 2, T], b16, name=f"xt{dp}")
                nc.sync.dma_start(
                    out=xt.rearrange("p a b -> p (a b)"),
                    in_=xT_d[:, dp * 2 * T : (dp + 1) * 2 * T],
                )
                xts.append(xt)

            def xchunk(dt, c0, c1):
                return xts[dt // 2][:, dt % 2, c0:c1]

            # KT/QT[col, t] = w.T @ x (transposed orientation).  Loop qc inner
            # so each stationary w chunk serves T//512 matmuls.
            for w_d, T_sb, w_sb0, pname in (
                (None, KT_sb, wk_sb, "pwk"),
                (wq_d, QT_sb, None, "pwq"),
            ):
                with (
                    tc.tile_pool(name=pname + "w", bufs=1) as pwx,
                    tc.tile_pool(name=pname + "j", bufs=2 * NQ5, space="PSUM") as cprj,
                ):
                    if w_sb0 is None:
                        w_sb = pwx.tile([128, DT, CD], b16, name=pname + "t")
                        nc.sync.dma_start(out=w_sb.rearrange("p a b -> p (a b)"), in_=w_d[:])
                    else:
                        w_sb = w_sb0
                    for ct in range(CT):
                        c_ps = [
                            cprj.tile([128, 512], f32, tag="cps", name=f"{pname}ps{ct}{qc}")
                            for qc in range(NQ5)
                        ]
                        for dt in range(DT):
                            for qc in range(NQ5):
                                nc.tensor.matmul(
                                    c_ps[qc][:],
                                    w_sb[:, dt, ct * 128 : (ct + 1) * 128],
                                    xchunk(dt, qc * 512, (qc + 1) * 512),
                                    start=(dt == 0),
                                    stop=(dt == DT - 1),
                                )
                        for qc in range(NQ5):
                            nc.scalar.copy(T_sb[:, ct, qc * 512 : (qc + 1) * 512], c_ps[qc][:])

            # V[t, vcol] = x @ wv  (normal orientation: kidx on partitions)
            with (
                tc.tile_pool(name="pwv", bufs=1) as pwv,
                tc.tile_pool(name="vprj", bufs=2, space="PSUM") as vprj,
            ):
                wv_sb = pwv.tile([128, DT, CD], b16)
                nc.sync.dma_start(out=wv_sb.rearrange("p a b -> p (a b)"), in_=wv_d[:])
                for t in range(NT):
                    v_ps = vprj.tile([128, CD], f32, tag="vps")
                    for dt in range(DT):
                        nc.tensor.matmul(
                            v_ps[:],
                            xchunk(dt, t * 128, (t + 1) * 128),
                            wv_sb[:, dt, :],
                            start=(dt == 0),
                            stop=(dt == DT - 1),
                        )
                    Vv = V_sb.rearrange("p t (c w) -> p t c w", w=PB)
                    pv = v_ps.rearrange("p (c two h) -> p c two h", two=2, h=HD)
                    nc.vector.tensor_copy(Vv[:, t, :, 0:HD], pv[:, :, 0, :])
                    nc.vector.tensor_copy(Vv[:, t, :, 2 * HD : PB], pv[:, :, 1, :])

        # ---- attention (xT / w pools released) ----
        poT = ctx.enter_context(tc.tile_pool(name="poT", bufs=1))
        oT_sb = poT.tile([128, CT, T], b16)
        # prefetch the projection weights during attention
        pwp = ctx.enter_context(tc.tile_pool(name="pwp", bufs=1))
        wp_sb = pwp.tile([128, CT, D], b16)
        nc.sync.dma_start(out=wp_sb.rearrange("p a b -> p (a b)"), in_=wp_d[:])

        with (
            tc.tile_pool(name="pexp", bufs=6) as pexp,
            tc.tile_pool(name="pnrm", bufs=3) as pnrm,
            tc.tile_pool(name="pdr", bufs=6, space="DRAM") as pdr,
            tc.tile_pool(name="sps", bufs=2, space="PSUM") as sps,
            tc.tile_pool(name="aps", bufs=2 * NQC, space="PSUM") as aps,
        ):
            for p_i in range(CT):
                for hh in range(2):
                    QTh = QT_sb[hh * 64 : (hh + 1) * 64, p_i, :]
                    KTh = KT_sb[hh * 64 : (hh + 1) * 64, p_i, :]
                    # ones sits at J col HD//2: sums row = HD + HD//2 (even
                    # head, lhsT=[V_h|J]) or HD//2 (odd head, lhsT=[J|V_h])
                    sum_row = HD // 2 if hh else HD + HD // 2
                    out_lo = HD if hh else 0
                    vbase = HD if hh else 0
                    for half in range(2):
                        q0 = half * HW
                        kt_hi = (q0 + HW) // 128
                        accs = [
                            aps.tile([128, 512], f32, name=f"acc{p_i}{hh}{half}{i}", tag="acc")
                            for i in range(NQC)
                        ]

                        def emit_pv(kt, ex, lo):
                            lhsT = V_sb[:, kt, p_i * PB + vbase : p_i * PB + vbase + 2 * HD]
                            for qcl in range(NQC):
                                rlo = max(lo - qcl * 512, 0)
                                if rlo >= 512:
                                    continue  # block fully above this q chunk
                                last_kt = (q0 + (qcl + 1) * 512) // 128 - 1
                                nc.tensor.matmul(
                                    accs[qcl][:, rlo:],
                                    lhsT,
                                    ex[:, qcl * 512 + rlo : (qcl + 1) * 512],
                                    start=(kt == 0),
                                    stop=(kt == last_kt),
                                )

                        # PV trails S/exp by 2 kt steps so the PE never waits
                        # on the scalar engine's exp
                        pend = []
                        for kt in range(kt_hi):
                            lo = max(kt * 128 - q0, 0)
                            s_ps = sps.tile([128, HW], f32, tag="sps")
                            c = lo
                            while c < HW:
                                c1 = min((c // 512 + 1) * 512, HW)
                                nc.tensor.matmul(
                                    s_ps[:, c:c1],
                                    KTh[:, kt * 128 : (kt + 1) * 128],
                                    QTh[:, q0 + c : q0 + c1],
                                    start=True,
                                    stop=True,
                                )
                                c = c1
                            ex = pexp.tile([128, HW], b16, tag="ex")
                            nc.scalar.activation(ex[:, lo:], s_ps[:, lo:], EXP, scale=scale)
                            if kt * 128 >= q0:  # diagonal block lives in this half
                                dl = kt * 128 - q0
                                nc.vector.tensor_mul(ex[:, dl : dl + 128], ex[:, dl : dl + 128], utri[:])
                            pend.append((kt, ex, lo))
                            if len(pend) > 2:
                                emit_pv(*pend.pop(0))
                        for e in pend:
                            emit_pv(*e)

                        # normalize: out^T[d, q] * (1 / sums[q]).  Reciprocal
                        # is done in a [128, HW/128] reshaped tile mid-bounce
                        # (the DVE divide is 8 cyc/elem; spread it across all
                        # lanes), then broadcast to the out rows via a second
                        # DRAM bounce.
                        rb_row = pnrm.tile([1, HW], f32, name=f"rr{p_i}{hh}{half}", tag="rbrow")
                        for qcl in range(NQC):
                            nc.vector.tensor_copy(
                                rb_row[0:1, qcl * 512 : (qcl + 1) * 512],
                                accs[qcl][sum_row : sum_row + 1, :],
                            )
                        scr = pdr.tile([1, HW], f32, name=f"scr{p_i}{hh}{half}", tag="scr")
                        nc.sync.dma_start(out=scr[:], in_=rb_row[0:1, :])
                        rsh = pnrm.tile([128, HWP], f32, name=f"rs{p_i}{hh}{half}", tag="rsh")
                        scr_sq = bass.AP(tensor=scr.tensor, offset=scr.offset, ap=[[HWP, 128], [1, HWP]])
                        nc.sync.dma_start(out=rsh[:], in_=scr_sq)
                        nc.vector.reciprocal(rsh[:], rsh[:])
                        scr2 = pdr.tile([1, HW], f32, name=f"sc2{p_i}{hh}{half}", tag="scr2")
                        scr2_sq = bass.AP(tensor=scr2.tensor, offset=scr2.offset, ap=[[HWP, 128], [1, HWP]])
                        nc.sync.dma_start(out=scr2_sq, in_=rsh[:])
                        rb = pnrm.tile([128, HW], f32, name=f"rb{p_i}{hh}{half}", tag="rb")
                        sbc = bass.AP(tensor=scr2.tensor, offset=scr2.offset, ap=[[0, HD], [1, HW]])
                        nc.sync.dma_start(out=rb[out_lo : out_lo + HD, :], in_=sbc)
                        for qcl in range(NQC):
                            nc.vector.tensor_mul(
                                oT_sb[out_lo : out_lo + HD, p_i, q0 + qcl * 512 : q0 + (qcl + 1) * 512],
                                accs[qcl][out_lo : out_lo + HD, :],
                                rb[out_lo : out_lo + HD, qcl * 512 : (qcl + 1) * 512],
                            )

        # ---- output projection: oT = wp.T @ out^T ----
        with (
            tc.tile_pool(name="ppo", bufs=3) as ppo,
            tc.tile_pool(name="pps", bufs=6, space="PSUM") as pps,
        ):
            for nt in range(D // 128):
                p_ps = [
                    pps.tile([128, 512], f32, tag="pps", name=f"pps{nt}{qc}")
                    for qc in range(NQ5)
                ]
                for ct in range(CT):
                    for qc in range(NQ5):
                        nc.tensor.matmul(
                            p_ps[qc][:],
                            wp_sb[:, ct, nt * 128 : (nt + 1) * 128],
                            oT_sb[:, ct, qc * 512 : (qc + 1) * 512],
                            start=(ct == 0),
                            stop=(ct == CT - 1),
                        )
                po = ppo.tile([128, T], b16, tag="po", name=f"po{nt}")
                for qc in range(NQ5):
                    nc.vector.tensor_copy(po[:, qc * 512 : (qc + 1) * 512], p_ps[qc][:])
                nc.sync.dma_start(out=oT_d[nt * 128 : (nt + 1) * 128, :], in_=po[:])

    nc.compile()
    return nc


def _get_compiled():
    key = (T, D, HC, HD)
    if key not in _CACHE:
        _CACHE[key] = build_attention_kernel(*key)
    return _CACHE[key]


def _tile128(a, chunk):
    """[R, C] -> [128, (R//128)*C] with row-block-major layout."""
    R, C = a.shape
    return np.ascontiguousarray(
        a.reshape(R // 128, 128, C).transpose(1, 0, 2).reshape(128, (R // 128) * C)
    )


def shard_inputs(x, w_qkv, w_proj):
    import ml_dtypes

    bf = ml_dtypes.bfloat16
    jc = np.zeros((128, HD), np.float32)
    jc[:, HD // 2] = 1.0
    jc = jc.astype(bf)
    ut = np.triu(np.ones((128, 128), np.float32)).astype(bf)
    in_maps = []
    for c in range(N_CORES):
        b, g = c // HG, c % HG
        in_maps.append(
            dict(
                jc=jc,
                ut=ut,
                xT=_tile128(x[b].T.astype(bf), T),
                wq=_tile128(w_qkv[:, g * CD : (g + 1) * CD].astype(bf), CD),
                wk=_tile128(w_qkv[:, D + g * CD : D + (g + 1) * CD].astype(bf), CD),
                wv=_tile128(w_qkv[:, 2 * D + g * CD : 2 * D + (g + 1) * CD].astype(bf), CD),
                wp=_tile128(w_proj[g * CD : (g + 1) * CD, :].astype(bf), D),
            )
        )
    return in_maps


def gather_outputs(results):
    out = np.empty((B, T, D), np.float32)
    for b in range(B):
        acc = results[HG * b]["oT"].astype(np.float32) + results[HG * b + 1]["oT"].astype(np.float32)
        out[b] = acc.T
    return out


def kernel(x, w_qkv, w_proj):
    from concourse.bass_utils import run_bass_kernel_spmd

    x = np.asarray(x, dtype=np.float32)
    w_qkv = np.asarray(w_qkv, dtype=np.float32)
    w_proj = np.asarray(w_proj, dtype=np.float32)
    nc = _get_compiled()
    res = run_bass_kernel_spmd(nc, shard_inputs(x, w_qkv, w_proj), list(range(N_CORES)))
    return gather_outputs(res.results)


# revision 20
# speedup vs baseline: 1.2026x; 1.0011x over previous
"""Causal self-attention on 8 Trainium2 NeuronCores (Bass/Tile), bf16.

Sharding: core c -> (batch b = c//2, head-group g = c%2).  Each core runs
attention for 8 heads of one batch element: qkv projection (columns of
w_qkv for its heads), causal softmax attention, and its half of the
output projection (rows of w_proj).  Host sums the two partial
projections per batch and transposes back.

Dataflow is fully transposed on-device (the contraction dim always sits
on SBUF partitions, so no on-device transposes are needed anywhere):
  xT [D, T] -> qT/kT [64, T] per head -> S^T [kidx, q] blocks -> exp ->
  PV gives out^T [d, q]; an appended ones-column in the stationary
  operand makes the PE produce the softmax denominators for free ->
  w_proj consumed natively as lhsT -> oT [D, T] partial output.

All SBUF data and DMA traffic is bf16 (PSUM accumulation stays fp32);
the host casts inputs and pre-tiles them to [128, ...] so each weight
loads in a single large DMA (HWDGE issue cost is ~0.6us per dma_start).

V layout per kidx-tile and head pair: [V_even | J | V_odd] where J is
64 cols of zeros with a 1.0 in col 32.  PV stationary for an even head
is [V_h | J] (out rows 0-63, sums row 96), for an odd head [J | V_h]
(sums row 32, out rows 64-127) -- the appended ones-column makes the PE
produce the softmax denominators in the same pass as PV, and keeps all
later elementwise ops partition- and 32-aligned.

Softmax skips max-subtraction (scores ~ N(0,1); exp is safe in fp32)
and causal-skips fully-masked blocks in scores, exp, and PV.  The
denominators are reciprocal'd in a [128, HW/128] reshaped tile
mid-DRAM-bounce so the DVE's 8-cycle/element iterative divide runs
128-lane-parallel, then broadcast to the out rows via a second DRAM
bounce (SBUF DMA cannot replicate partitions).
"""

import sys

sys.path.insert(0, "/opt/trn_rl_repo")

import numpy as np

N_CORES = 8
B, T, D = 4, 2048, 1024
H, HD = 16, 64
HG = 2  # head groups (tensor parallel)
HC = H // HG  # heads per core
CD = HC * HD  # per-core qkv width (512)

_CACHE = {}


def build_attention_kernel(T, D, HC, HD, n_cores=N_CORES):
    """Build + compile the per-core Bass module (same module on all cores)."""
    from contextlib import ExitStack

    import concourse.bass as bass
    import concourse.mybir as mybir
    import concourse.tile as tile
    from concourse import bacc

    f32 = mybir.dt.float32
    b16 = mybir.dt.bfloat16
    EXP = mybir.ActivationFunctionType.Exp

    CD = HC * HD
    DT = D // 128  # contraction chunks over D
    NT = T // 128  # T tiles / kidx tiles
    CT = CD // 128  # col tiles of q/k block (= head pairs)
    HW = T // 2  # q-half width
    HWP = HW // 128  # reshaped denominator cols per partition
    NQC = HW // 512  # 512-wide q chunks per half
    PB = 3 * HD  # V_sb block width per head pair: [V_even | J | V_odd]
    VW = CT * PB  # V_sb row width
    NQ5 = T // 512
    scale = 1.0 / float(np.sqrt(HD))

    nc = bacc.Bacc("TRN2", target_bir_lowering=False, debug=False, num_devices=n_cores)

    xT_d = nc.dram_tensor("xT", [128, DT * T], b16, kind="ExternalInput").ap()
    wq_d = nc.dram_tensor("wq", [128, DT * CD], b16, kind="ExternalInput").ap()
    wk_d = nc.dram_tensor("wk", [128, DT * CD], b16, kind="ExternalInput").ap()
    wv_d = nc.dram_tensor("wv", [128, DT * CD], b16, kind="ExternalInput").ap()
    wp_d = nc.dram_tensor("wp", [128, CT * D], b16, kind="ExternalInput").ap()
    jc_d = nc.dram_tensor("jc", [128, HD], b16, kind="ExternalInput").ap()
    ut_d = nc.dram_tensor("ut", [128, 128], b16, kind="ExternalInput").ap()
    oT_d = nc.dram_tensor("oT", [D, T], b16, kind="ExternalOutput").ap()

    with tile.TileContext(nc) as tc, ExitStack() as ctx:
        # ---- persistent SBUF ----
        pers = ctx.enter_context(tc.tile_pool(name="pers", bufs=1))
        V_sb = pers.tile([128, NT, VW], b16)
        QT_sb = pers.tile([128, CT, T], b16)
        KT_sb = pers.tile([128, CT, T], b16)
        utri = pers.tile([128, 128], b16)
        nc.sync.dma_start(out=utri[:], in_=ut_d[:])

        J_sb = pers.tile([128, HD], b16)
        nc.sync.dma_start(out=J_sb[:], in_=jc_d[:])
        for kt in range(NT):
            for p_i in range(CT):
                nc.vector.tensor_copy(
                    V_sb[:, kt, p_i * PB + HD : p_i * PB + 2 * HD], J_sb[:]
                )

        # ---- phase K/Q/V: projections of x (xT resident only here) ----
        with (
            tc.tile_pool(name="pwk", bufs=1) as pwk,
            tc.tile_pool(name="px", bufs=1) as px,
        ):
            wk_sb = pwk.tile([128, DT, CD], b16)
            nc.sync.dma_start(out=wk_sb.rearrange("p a b -> p (a b)"), in_=wk_d[:])
            # xT in dt-pair tiles so the first matmuls only wait on the
            # first 1 MB, not the full 4 MB
            xts = []
            for dp in range(DT // 2):
                xt = px.tile([128, 2, T], b16, name=f"xt{dp}")
                nc.sync.dma_start(
                    out=xt.rearrange("p a b -> p (a b)"),
                    in_=xT_d[:, dp * 2 * T : (dp + 1) * 2 * T],
                )
                xts.append(xt)

            def xchunk(dt, c0, c1):
                return xts[dt // 2][:, dt % 2, c0:c1]

            # KT/QT[col, t] = w.T @ x (transposed orientation).  Loop qc inner
            # so each stationary w chunk serves T//512 matmuls.
            for w_d, T_sb, w_sb0, pname in (
                (None, KT_sb, wk_sb, "pwk"),
                (wq_d, QT_sb, None, "pwq"),
            ):
                with (
                    tc.tile_pool(name=pname + "w", bufs=1) as pwx,
                    tc.tile_pool(name=pname + "j", bufs=2 * NQ5, space="PSUM") as cprj,
                ):
                    if w_sb0 is None:
                        w_sb = pwx.tile([128, DT, CD], b16, name=pname + "t")
                        nc.sync.dma_start(out=w_sb.rearrange("p a b -> p (a b)"), in_=w_d[:])
                    else:
                        w_sb = w_sb0
                    for ct in range(CT):
                        c_ps = [
                            cprj.tile([128, 512], f32, tag="cps", name=f"{pname}ps{ct}{qc}")
                            for qc in range(NQ5)
                        ]
                        for dt in range(DT):
                            for qc in range(NQ5):
                                nc.tensor.matmul(
                                    c_ps[qc][:],
                                    w_sb[:, dt, ct * 128 : (ct + 1) * 128],
                                    xchunk(dt, qc * 512, (qc + 1) * 512),
                                    start=(dt == 0),
                                    stop=(dt == DT - 1),
                                )
                        for qc in range(NQ5):
                            nc.scalar.copy(T_sb[:, ct, qc * 512 : (qc + 1) * 512], c_ps[qc][:])

            # V[t, vcol] = x @ wv  (normal orientation: kidx on partitions)
            with (
                tc.tile_pool(name="pwv", bufs=1) as pwv,
                tc.tile_pool(name="vprj", bufs=2, space="PSUM") as vprj,
            ):
                wv_sb = pwv.tile([128, DT, CD], b16)
                nc.sync.dma_start(out=wv_sb.rearrange("p a b -> p (a b)"), in_=wv_d[:])
                for t in range(NT):
                    v_ps = vprj.tile([128, CD], f32, tag="vps")
                    for dt in range(DT):
                        nc.tensor.matmul(
                            v_ps[:],
                            xchunk(dt, t * 128, (t + 1) * 128),
                            wv_sb[:, dt, :],
                            start=(dt == 0),
                            stop=(dt == DT - 1),
                        )
                    Vv = V_sb.rearrange("p t (c w) -> p t c w", w=PB)
                    pv = v_ps.rearrange("p (c two h) -> p c two h", two=2, h=HD)
                    nc.vector.tensor_copy(Vv[:, t, :, 0:HD], pv[:, :, 0, :])
                    nc.vector.tensor_copy(Vv[:, t, :, 2 * HD : PB], pv[:, :, 1, :])

        # ---- attention (xT / w pools released) ----
        poT = ctx.enter_context(tc.tile_pool(name="poT", bufs=1))
        oT_sb = poT.tile([128, CT, T], b16)
        # prefetch the projection weights during attention
        pwp = ctx.enter_context(tc.tile_pool(name="pwp", bufs=1))
        wp_sb = pwp.tile([128, CT, D], b16)
        nc.sync.dma_start(out=wp_sb.rearrange("p a b -> p (a b)"), in_=wp_d[:])

        with (
            tc.tile_pool(name="pexp", bufs=6) as pexp,
            tc.tile_pool(name="pnrm", bufs=3) as pnrm,
            tc.tile_pool(name="pdr", bufs=6, space="DRAM") as pdr,
            tc.tile_pool(name="sps", bufs=2, space="PSUM") as sps,
            tc.tile_pool(name="aps", bufs=2 * NQC, space="PSUM") as aps,
        ):
            for p_i in range(CT):
                for hh in range(2):
                    QTh = QT_sb[hh * 64 : (hh + 1) * 64, p_i, :]
                    KTh = KT_sb[hh * 64 : (hh + 1) * 64, p_i, :]
                    # ones sits at J col HD//2: sums row = HD + HD//2 (even
                    # head, lhsT=[V_h|J]) or HD//2 (odd head, lhsT=[J|V_h])
                    sum_row = HD // 2 if hh else HD + HD // 2
                    out_lo = HD if hh else 0
                    vbase = HD if hh else 0
                    for half in range(2):
                        q0 = half * HW
                        kt_hi = (q0 + HW) // 128
                        accs = [
                            aps.tile([128, 512], f32, name=f"acc{p_i}{hh}{half}{i}", tag="acc")
                            for i in range(NQC)
                        ]

                        def emit_pv(kt, ex, lo):
                            lhsT = V_sb[:, kt, p_i * PB + vbase : p_i * PB + vbase + 2 * HD]
                            for qcl in range(NQC):
                                rlo = max(lo - qcl * 512, 0)
                                if rlo >= 512:
                                    continue  # block fully above this q chunk
                                last_kt = (q0 + (qcl + 1) * 512) // 128 - 1
                                nc.tensor.matmul(
                                    accs[qcl][:, rlo:],
                                    lhsT,
                                    ex[:, qcl * 512 + rlo : (qcl + 1) * 512],
                                    start=(kt == 0),
                                    stop=(kt == last_kt),
                                )

                        # PV trails S/exp by 2 kt steps so the PE never waits
                        # on the scalar engine's exp
                        pend = []
                        for kt in range(kt_hi):
                            lo = max(kt * 128 - q0, 0)
                            s_ps = sps.tile([128, HW], f32, tag="sps")
                            c = lo
                            while c < HW:
                                c1 = min((c // 512 + 1) * 512, HW)
                                nc.tensor.matmul(
                                    s_ps[:, c:c1],
                                    KTh[:, kt * 128 : (kt + 1) * 128],
                                    QTh[:, q0 + c : q0 + c1],
                                    start=True,
                                    stop=True,
                                )
                                c = c1
                            ex = pexp.tile([128, HW], b16, tag="ex")
                            nc.scalar.activation(ex[:, lo:], s_ps[:, lo:], EXP, scale=scale)
                            if kt * 128 >= q0:  # diagonal block lives in this half
                                dl = kt * 128 - q0
                                nc.vector.tensor_mul(ex[:, dl : dl + 128], ex[:, dl : dl + 128], utri[:])
                            pend.append((kt, ex, lo))
                            if len(pend) > 2:
                                emit_pv(*pend.pop(0))
                        for e in pend:
                            emit_pv(*e)

                        # normalize: out^T[d, q] * (1 / sums[q]).  Reciprocal
                        # is done in a [128, HW/128] reshaped tile mid-bounce
                        # (the DVE divide is 8 cyc/elem; spread it across all
                        # lanes), then broadcast to the out rows via a second
                        # DRAM bounce.
                        rb_row = pnrm.tile([1, HW], f32, name=f"rr{p_i}{hh}{half}", tag="rbrow")
                        for qcl in range(NQC):
                            nc.vector.tensor_copy(
                                rb_row[0:1, qcl * 512 : (qcl + 1) * 512],
                                accs[qcl][sum_row : sum_row + 1, :],
                            )
                        scr = pdr.tile([1, HW], f32, name=f"scr{p_i}{hh}{half}", tag="scr")
                        nc.sync.dma_start(out=scr[:], in_=rb_row[0:1, :])
                        rsh = pnrm.tile([128, HWP], f32, name=f"rs{p_i}{hh}{half}", tag="rsh")
                        scr_sq = bass.AP(tensor=scr.tensor, offset=scr.offset, ap=[[HWP, 128], [1, HWP]])
                        nc.sync.dma_start(out=rsh[:], in_=scr_sq)
                        nc.vector.reciprocal(rsh[:], rsh[:])
                        scr2 = pdr.tile([1, HW], f32, name=f"sc2{p_i}{hh}{half}", tag="scr2")
                        scr2_sq = bass.AP(tensor=scr2.tensor, offset=scr2.offset, ap=[[HWP, 128], [1, HWP]])
                        nc.sync.dma_start(out=scr2_sq, in_=rsh[:])
                        rb = pnrm.tile([128, HW], f32, name=f"rb{p_i}{hh}{half}", tag="rb")
                        sbc = bass.AP(tensor=scr2.tensor, offset=scr2.offset, ap=[[0, HD], [1, HW]])
                        nc.sync.dma_start(out=rb[out_lo : out_lo + HD, :], in_=sbc)
                        for qcl in range(NQC):
                            nc.vector.tensor_mul(
                                oT_sb[out_lo : out_lo + HD, p_i, q0 + qcl * 512 : q0 + (qcl + 1) * 512],
                                accs[qcl][out_lo : out_lo + HD, :],
                                rb[out_lo : out_lo + HD, qcl * 512 : (qcl + 1) * 512],
                            )

        # ---- output projection: oT = wp.T @ out^T ----
        with (
            tc.tile_pool(name="ppo", bufs=3) as ppo,
            tc.tile_pool(name="pps", bufs=6, space="PSUM") as pps,
        ):
            for nt in range(D // 128):
                p_ps = [
                    pps.tile([128, 512], f32, tag="pps", name=f"pps{nt}{qc}")
                    for qc in range(NQ5)
                ]
                for ct in range(CT):
                    for qc in range(NQ5):
                        nc.tensor.matmul(
                            p_ps[qc][:],
                            wp_sb[:, ct, nt * 128 : (nt + 1) * 128],
                            oT_sb[:, ct, qc * 512 : (qc + 1) * 512],
                            start=(ct == 0),
                            stop=(ct == CT - 1),
                        )
                po = ppo.tile([128, T], b16, tag="po", name=f"po{nt}")
                for qc in range(NQ5):
                    nc.vector.tensor_copy(po[:, qc * 512 : (qc + 1) * 512], p_ps[qc][:])
                nc.sync.dma_start(out=oT_d[nt * 128 : (nt + 1) * 128, :], in_=po[:])

    nc.compile()
    return nc


def _get_compiled():
    key = (T, D, HC, HD)
    if key not in _CACHE:
        _CACHE[key] = build_attention_kernel(*key)
    return _CACHE[key]


def _tile128(a, chunk):
    """[R, C] -> [128, (R//128)*C] with row-block-major layout."""
    R, C = a.shape
    return np.ascontiguousarray(
        a.reshape(R // 128, 128, C).transpose(1, 0, 2).reshape(128, (R // 128) * C)
    )


def shard_inputs(x, w_qkv, w_proj):
    import ml_dtypes

    bf = ml_dtypes.bfloat16
    jc = np.zeros((128, HD), np.float32)
    jc[:, HD // 2] = 1.0
    jc = jc.astype(bf)
    ut = np.triu(np.ones((128, 128), np.float32)).astype(bf)
    in_maps = []
    for c in range(N_CORES):
        b, g = c // HG, c % HG
        in_maps.append(
            dict(
                jc=jc,
                ut=ut,
                xT=_tile128(x[b].T.astype(bf), T),
                wq=_tile128(w_qkv[:, g * CD : (g + 1) * CD].astype(bf), CD),
                wk=_tile128(w_qkv[:, D + g * CD : D + (g + 1) * CD].astype(bf), CD),
                wv=_tile128(w_qkv[:, 2 * D + g * CD : 2 * D + (g + 1) * CD].astype(bf), CD),
                wp=_tile128(w_proj[g * CD : (g + 1) * CD, :].astype(bf), D),
            )
        )
    return in_maps


def gather_outputs(results):
    out = np.empty((B, T, D), np.float32)
    for b in range(B):
        acc = results[HG * b]["oT"].astype(np.float32) + results[HG * b + 1]["oT"].astype(np.float32)
        out[b] = acc.T
    return out


def kernel(x, w_qkv, w_proj):
    from concourse.bass_utils import run_bass_kernel_spmd

    x = np.asarray(x, dtype=np.float32)
    w_qkv = np.asarray(w_qkv, dtype=np.float32)
    w_proj = np.asarray(w_proj, dtype=np.float32)
    nc = _get_compiled()
    res = run_bass_kernel_spmd(nc, shard_inputs(x, w_qkv, w_proj), list(range(N_CORES)))
    return gather_outputs(res.results)


# revision 21
# speedup vs baseline: 1.2228x; 1.0168x over previous
"""Causal self-attention on 8 Trainium2 NeuronCores (Bass/Tile), bf16.

Sharding: core c -> (batch b = c//2, head-group g = c%2).  Each core runs
attention for 8 heads of one batch element: qkv projection (columns of
w_qkv for its heads), causal softmax attention, and its half of the
output projection (rows of w_proj).  Host sums the two partial
projections per batch and transposes back.

Dataflow is fully transposed on-device (the contraction dim always sits
on SBUF partitions, so no on-device transposes are needed anywhere):
  xT [D, T] -> qT/kT [64, T] per head -> S^T [kidx, q] blocks -> exp ->
  PV gives out^T [d, q]; an appended ones-column in the stationary
  operand makes the PE produce the softmax denominators for free ->
  w_proj consumed natively as lhsT -> oT [D, T] partial output.

All SBUF data and DMA traffic is bf16 (PSUM accumulation stays fp32);
the host casts inputs and pre-tiles them to [128, ...] so each weight
loads in a single large DMA (HWDGE issue cost is ~0.6us per dma_start).

V layout per kidx-tile and head pair: [V_even | J | V_odd] where J is
64 cols of zeros with a 1.0 in col 32.  PV stationary for an even head
is [V_h | J] (out rows 0-63, sums row 96), for an odd head [J | V_h]
(sums row 32, out rows 64-127) -- the appended ones-column makes the PE
produce the softmax denominators in the same pass as PV, and keeps all
later elementwise ops partition- and 32-aligned.

Softmax skips max-subtraction (scores ~ N(0,1); exp is safe in fp32)
and causal-skips fully-masked blocks in scores, exp, and PV.  The
denominators are reciprocal'd in a [128, HW/128] reshaped tile
mid-DRAM-bounce so the DVE's 8-cycle/element iterative divide runs
128-lane-parallel, then broadcast to the out rows via a second DRAM
bounce (SBUF DMA cannot replicate partitions).
"""

import sys

sys.path.insert(0, "/opt/trn_rl_repo")

import numpy as np

N_CORES = 8
B, T, D = 4, 2048, 1024
H, HD = 16, 64
HG = 2  # head groups (tensor parallel)
HC = H // HG  # heads per core
CD = HC * HD  # per-core qkv width (512)

_CACHE = {}


def build_attention_kernel(T, D, HC, HD, n_cores=N_CORES):
    """Build + compile the per-core Bass module (same module on all cores)."""
    from contextlib import ExitStack

    import concourse.bass as bass
    import concourse.mybir as mybir
    import concourse.tile as tile
    from concourse import bacc

    f32 = mybir.dt.float32
    b16 = mybir.dt.bfloat16
    EXP = mybir.ActivationFunctionType.Exp

    CD = HC * HD
    DT = D // 128  # contraction chunks over D
    NT = T // 128  # T tiles / kidx tiles
    CT = CD // 128  # col tiles of q/k block (= head pairs)
    HW = T // 2  # q-half width
    HWP = HW // 128  # reshaped denominator cols per partition
    NQC = HW // 512  # 512-wide q chunks per half
    PB = 3 * HD  # V_sb block width per head pair: [V_even | J | V_odd]
    VW = CT * PB  # V_sb row width
    NQ5 = T // 512
    scale = 1.0 / float(np.sqrt(HD))

    nc = bacc.Bacc("TRN2", target_bir_lowering=False, debug=False, num_devices=n_cores)

    xT_d = nc.dram_tensor("xT", [128, DT * T], b16, kind="ExternalInput").ap()
    wq_d = nc.dram_tensor("wq", [128, DT * CD], b16, kind="ExternalInput").ap()
    wk_d = nc.dram_tensor("wk", [128, DT * CD], b16, kind="ExternalInput").ap()
    wv_d = nc.dram_tensor("wv", [128, DT * CD], b16, kind="ExternalInput").ap()
    wp_d = nc.dram_tensor("wp", [128, CT * D], b16, kind="ExternalInput").ap()
    jc_d = nc.dram_tensor("jc", [128, HD], b16, kind="ExternalInput").ap()
    ut_d = nc.dram_tensor("ut", [128, 128], b16, kind="ExternalInput").ap()
    oT_d = nc.dram_tensor("oT", [D, T], b16, kind="ExternalOutput").ap()

    with tile.TileContext(nc) as tc, ExitStack() as ctx:
        # ---- persistent SBUF ----
        pers = ctx.enter_context(tc.tile_pool(name="pers", bufs=1))
        V_sb = pers.tile([128, NT, VW], b16)
        QT_sb = pers.tile([128, CT, T], b16)
        KT_sb = pers.tile([128, CT, T], b16)
        utri = pers.tile([128, 128], b16)
        nc.sync.dma_start(out=utri[:], in_=ut_d[:])

        J_sb = pers.tile([128, HD], b16)
        nc.sync.dma_start(out=J_sb[:], in_=jc_d[:])
        for kt in range(NT):
            for p_i in range(CT):
                nc.vector.tensor_copy(
                    V_sb[:, kt, p_i * PB + HD : p_i * PB + 2 * HD], J_sb[:]
                )

        # ---- phase K/Q/V: projections of x (xT resident only here) ----
        with (
            tc.tile_pool(name="pwk", bufs=1) as pwk,
            tc.tile_pool(name="px", bufs=1) as px,
        ):
            wk_sb = pwk.tile([128, DT, CD], b16)
            nc.sync.dma_start(out=wk_sb.rearrange("p a b -> p (a b)"), in_=wk_d[:])
            # xT in dt-pair tiles so the first matmuls only wait on the
            # first 1 MB, not the full 4 MB
            xts = []
            for dp in range(DT // 2):
                xt = px.tile([128, 2, T], b16, name=f"xt{dp}")
                nc.sync.dma_start(
                    out=xt.rearrange("p a b -> p (a b)"),
                    in_=xT_d[:, dp * 2 * T : (dp + 1) * 2 * T],
                )
                xts.append(xt)

            def xchunk(dt, c0, c1):
                return xts[dt // 2][:, dt % 2, c0:c1]

            # KT/QT[col, t] = w.T @ x (transposed orientation).  Loop qc inner
            # so each stationary w chunk serves T//512 matmuls.
            for w_d, T_sb, w_sb0, pname in (
                (None, KT_sb, wk_sb, "pwk"),
                (wq_d, QT_sb, None, "pwq"),
            ):
                with (
                    tc.tile_pool(name=pname + "w", bufs=1) as pwx,
                    tc.tile_pool(name=pname + "j", bufs=2 * NQ5, space="PSUM") as cprj,
                ):
                    if w_sb0 is None:
                        w_sb = pwx.tile([128, DT, CD], b16, name=pname + "t")
                        nc.sync.dma_start(out=w_sb.rearrange("p a b -> p (a b)"), in_=w_d[:])
                    else:
                        w_sb = w_sb0
                    for ct in range(CT):
                        c_ps = [
                            cprj.tile([128, 512], f32, tag="cps", name=f"{pname}ps{ct}{qc}")
                            for qc in range(NQ5)
                        ]
                        for dt in range(DT):
                            for qc in range(NQ5):
                                nc.tensor.matmul(
                                    c_ps[qc][:],
                                    w_sb[:, dt, ct * 128 : (ct + 1) * 128],
                                    xchunk(dt, qc * 512, (qc + 1) * 512),
                                    start=(dt == 0),
                                    stop=(dt == DT - 1),
                                )
                        for qc in range(NQ5):
                            nc.scalar.copy(T_sb[:, ct, qc * 512 : (qc + 1) * 512], c_ps[qc][:])

            # V[t, vcol] = x @ wv  (normal orientation: kidx on partitions)
            with (
                tc.tile_pool(name="pwv", bufs=1) as pwv,
                tc.tile_pool(name="vprj", bufs=2, space="PSUM") as vprj,
            ):
                wv_sb = pwv.tile([128, DT, CD], b16)
                nc.sync.dma_start(out=wv_sb.rearrange("p a b -> p (a b)"), in_=wv_d[:])
                for t in range(NT):
                    v_ps = vprj.tile([128, CD], f32, tag="vps")
                    for dt in range(DT):
                        nc.tensor.matmul(
                            v_ps[:],
                            xchunk(dt, t * 128, (t + 1) * 128),
                            wv_sb[:, dt, :],
                            start=(dt == 0),
                            stop=(dt == DT - 1),
                        )
                    Vv = V_sb.rearrange("p t (c w) -> p t c w", w=PB)
                    pv = v_ps.rearrange("p (c two h) -> p c two h", two=2, h=HD)
                    nc.vector.tensor_copy(Vv[:, t, :, 0:HD], pv[:, :, 0, :])
                    nc.vector.tensor_copy(Vv[:, t, :, 2 * HD : PB], pv[:, :, 1, :])

        # ---- attention (xT / w pools released) ----
        poT = ctx.enter_context(tc.tile_pool(name="poT", bufs=1))
        oT_sb = poT.tile([128, CT, T], b16)
        # prefetch the projection weights during attention
        pwp = ctx.enter_context(tc.tile_pool(name="pwp", bufs=1))
        wp_sb = pwp.tile([128, CT, D], b16)
        nc.sync.dma_start(out=wp_sb.rearrange("p a b -> p (a b)"), in_=wp_d[:])

        with (
            tc.tile_pool(name="pexp", bufs=8) as pexp,
            tc.tile_pool(name="pnrm", bufs=3) as pnrm,
            tc.tile_pool(name="pdr", bufs=6, space="DRAM") as pdr,
            tc.tile_pool(name="sps", bufs=2, space="PSUM") as sps,
            tc.tile_pool(name="aps", bufs=2 * NQC, space="PSUM") as aps,
        ):
            for p_i in range(CT):
                for hh in range(2):
                    QTh = QT_sb[hh * 64 : (hh + 1) * 64, p_i, :]
                    KTh = KT_sb[hh * 64 : (hh + 1) * 64, p_i, :]
                    # ones sits at J col HD//2: sums row = HD + HD//2 (even
                    # head, lhsT=[V_h|J]) or HD//2 (odd head, lhsT=[J|V_h])
                    sum_row = HD // 2 if hh else HD + HD // 2
                    out_lo = HD if hh else 0
                    vbase = HD if hh else 0
                    for half in range(2):
                        q0 = half * HW
                        kt_hi = (q0 + HW) // 128
                        accs = [
                            aps.tile([128, 512], f32, name=f"acc{p_i}{hh}{half}{i}", tag="acc")
                            for i in range(NQC)
                        ]

                        def emit_pv(kt, ex, lo):
                            lhsT = V_sb[:, kt, p_i * PB + vbase : p_i * PB + vbase + 2 * HD]
                            for qcl in range(NQC):
                                rlo = max(lo - qcl * 512, 0)
                                if rlo >= 512:
                                    continue  # block fully above this q chunk
                                last_kt = (q0 + (qcl + 1) * 512) // 128 - 1
                                nc.tensor.matmul(
                                    accs[qcl][:, rlo:],
                                    lhsT,
                                    ex[:, qcl * 512 + rlo : (qcl + 1) * 512],
                                    start=(kt == 0),
                                    stop=(kt == last_kt),
                                )

                        # PV trails S/exp by 2 kt steps so the PE never waits
                        # on the scalar engine's exp
                        pend = []
                        for kt in range(kt_hi):
                            lo = max(kt * 128 - q0, 0)
                            s_ps = sps.tile([128, HW], f32, tag="sps")
                            c = lo
                            while c < HW:
                                c1 = min((c // 512 + 1) * 512, HW)
                                nc.tensor.matmul(
                                    s_ps[:, c:c1],
                                    KTh[:, kt * 128 : (kt + 1) * 128],
                                    QTh[:, q0 + c : q0 + c1],
                                    start=True,
                                    stop=True,
                                )
                                c = c1
                            ex = pexp.tile([128, HW], b16, tag="ex")
                            nc.scalar.activation(ex[:, lo:], s_ps[:, lo:], EXP, scale=scale)
                            if kt * 128 >= q0:  # diagonal block lives in this half
                                dl = kt * 128 - q0
                                nc.vector.tensor_mul(ex[:, dl : dl + 128], ex[:, dl : dl + 128], utri[:])
                            pend.append((kt, ex, lo))
                            if len(pend) > 3:
                                emit_pv(*pend.pop(0))
                        for e in pend:
                            emit_pv(*e)

                        # normalize: out^T[d, q] * (1 / sums[q]).  Reciprocal
                        # is done in a [128, HW/128] reshaped tile mid-bounce
                        # (the DVE divide is 8 cyc/elem; spread it across all
                        # lanes), then broadcast to the out rows via a second
                        # DRAM bounce.
                        rb_row = pnrm.tile([1, HW], f32, name=f"rr{p_i}{hh}{half}", tag="rbrow")
                        for qcl in range(NQC):
                            nc.vector.tensor_copy(
                                rb_row[0:1, qcl * 512 : (qcl + 1) * 512],
                                accs[qcl][sum_row : sum_row + 1, :],
                            )
                        scr = pdr.tile([1, HW], f32, name=f"scr{p_i}{hh}{half}", tag="scr")
                        nc.sync.dma_start(out=scr[:], in_=rb_row[0:1, :])
                        rsh = pnrm.tile([128, HWP], f32, name=f"rs{p_i}{hh}{half}", tag="rsh")
                        scr_sq = bass.AP(tensor=scr.tensor, offset=scr.offset, ap=[[HWP, 128], [1, HWP]])
                        nc.sync.dma_start(out=rsh[:], in_=scr_sq)
                        nc.vector.reciprocal(rsh[:], rsh[:])
                        scr2 = pdr.tile([1, HW], f32, name=f"sc2{p_i}{hh}{half}", tag="scr2")
                        scr2_sq = bass.AP(tensor=scr2.tensor, offset=scr2.offset, ap=[[HWP, 128], [1, HWP]])
                        nc.sync.dma_start(out=scr2_sq, in_=rsh[:])
                        rb = pnrm.tile([128, HW], f32, name=f"rb{p_i}{hh}{half}", tag="rb")
                        sbc = bass.AP(tensor=scr2.tensor, offset=scr2.offset, ap=[[0, HD], [1, HW]])
                        nc.sync.dma_start(out=rb[out_lo : out_lo + HD, :], in_=sbc)
                        for qcl in range(NQC):
                            nc.vector.tensor_mul(
                                oT_sb[out_lo : out_lo + HD, p_i, q0 + qcl * 512 : q0 + (qcl + 1) * 512],
                                accs[qcl][out_lo : out_lo + HD, :],
                                rb[out_lo : out_lo + HD, qcl * 512 : (qcl + 1) * 512],
                            )

        # ---- output projection: oT = wp.T @ out^T ----
        with (
            tc.tile_pool(name="ppo", bufs=3) as ppo,
            tc.tile_pool(name="pps", bufs=6, space="PSUM") as pps,
        ):
            for nt in range(D // 128):
                p_ps = [
                    pps.tile([128, 512], f32, tag="pps", name=f"pps{nt}{qc}")
                    for qc in range(NQ5)
                ]
                for ct in range(CT):
                    for qc in range(NQ5):
                        nc.tensor.matmul(
                            p_ps[qc][:],
                            wp_sb[:, ct, nt * 128 : (nt + 1) * 128],
                            oT_sb[:, ct, qc * 512 : (qc + 1) * 512],
                            start=(ct == 0),
                            stop=(ct == CT - 1),
                        )
                po = ppo.tile([128, T], b16, tag="po", name=f"po{nt}")
                for qc in range(NQ5):
                    nc.vector.tensor_copy(po[:, qc * 512 : (qc + 1) * 512], p_ps[qc][:])
                nc.sync.dma_start(out=oT_d[nt * 128 : (nt + 1) * 128, :], in_=po[:])

    nc.compile()
    return nc


def _get_compiled():
    key = (T, D, HC, HD)
    if key not in _CACHE:
        _CACHE[key] = build_attention_kernel(*key)
    return _CACHE[key]


def _tile128(a, chunk):
    """[R, C] -> [128, (R//128)*C] with row-block-major layout."""
    R, C = a.shape
    return np.ascontiguousarray(
        a.reshape(R // 128, 128, C).transpose(1, 0, 2).reshape(128, (R // 128) * C)
    )


def shard_inputs(x, w_qkv, w_proj):
    import ml_dtypes

    bf = ml_dtypes.bfloat16
    jc = np.zeros((128, HD), np.float32)
    jc[:, HD // 2] = 1.0
    jc = jc.astype(bf)
    ut = np.triu(np.ones((128, 128), np.float32)).astype(bf)
    in_maps = []
    for c in range(N_CORES):
        b, g = c // HG, c % HG
        in_maps.append(
            dict(
                jc=jc,
                ut=ut,
                xT=_tile128(x[b].T.astype(bf), T),
                wq=_tile128(w_qkv[:, g * CD : (g + 1) * CD].astype(bf), CD),
                wk=_tile128(w_qkv[:, D + g * CD : D + (g + 1) * CD].astype(bf), CD),
                wv=_tile128(w_qkv[:, 2 * D + g * CD : 2 * D + (g + 1) * CD].astype(bf), CD),
                wp=_tile128(w_proj[g * CD : (g + 1) * CD, :].astype(bf), D),
            )
        )
    return in_maps


def gather_outputs(results):
    out = np.empty((B, T, D), np.float32)
    for b in range(B):
        acc = results[HG * b]["oT"].astype(np.float32) + results[HG * b + 1]["oT"].astype(np.float32)
        out[b] = acc.T
    return out


def kernel(x, w_qkv, w_proj):
    from concourse.bass_utils import run_bass_kernel_spmd

    x = np.asarray(x, dtype=np.float32)
    w_qkv = np.asarray(w_qkv, dtype=np.float32)
    w_proj = np.asarray(w_proj, dtype=np.float32)
    nc = _get_compiled()
    res = run_bass_kernel_spmd(nc, shard_inputs(x, w_qkv, w_proj), list(range(N_CORES)))
    return gather_outputs(res.results)
